# revision 5
# baseline (speedup 1.0000x reference)
"""Trainium2 kernel for the NNUE-style factorized embedding segment-sum.

Strategy: the ragged two-table embedding-bag is reformulated as block-diagonal
dense matmuls.  For each output row (bag), the gather+segment-sum over its
ragged feature ids equals  counts_row @ table_block, where table_block is the
768-row slice of the merged factorized table selected by the bag's king square
(and counts columns are flip-remapped for the second output so only ONE table
is ever needed).  The factorization (tiles + (pieces+ranks+files)*mask) is kept
factorized: counts are extended with mask-weighted per-(k), per-(k,rank) and
per-(k,file) sums so the device contracts against the raw input tables and
never materializes the merged table.

Host (integer work only): build per-bag count rows, group (output,bag) items by
table block, shard blocks over 8 cores.  Device (all fp work): for each
128-item chunk, 8 accumulating matmuls (K=128, M=128, N=256) + clip to [0,1].

All floating point math happens on the NeuronCores.
"""

import numpy as np

import concourse.bass as bass
import concourse.tile as tile
from concourse import bacc, mybir
from concourse.bass_utils import run_bass_kernel_spmd

N_CORES = 8
B = 16384          # bags
KPL = 12           # piece planes
DOUT = 256
PIECE = 768        # KPL * 64
NFEAT = 972        # 768 tiles + 12 pieces + 96 ranks + 96 files
NFP = 1024         # padded features (8 chunks of 128)
NCHK = 8           # feature chunks per block
NBLK = 8           # table blocks per core (64 king squares / 8 cores)

# ---------------------------------------------------------------------------
# host-side integer prep tables
_sq = np.arange(64)
_PERM = (7 - _sq // 8) * 8 + _sq % 8          # vertical king-square flip
_v = np.arange(PIECE)
_vk, _vr, _vf = _v // 64, (_v % 64) // 8, _v % 8
_FLIP_COL = ((_vk + 6) % 12) * 64 + (7 - _vr) * 8 + _vf

_prog_cache = {}


def _build_program(cpb: int):
    """Bass program for one core: NCH=8*cpb chunks of 128 output rows."""
    nch = NBLK * cpb
    nc = bacc.Bacc("TRN2", target_bir_lowering=False, debug=False)
    f32r = mybir.dt.float32r
    f32 = mybir.dt.float32

    # tab[p, blk*NCHK*DOUT + j*DOUT + d] = table_block[blk][j*128+p, d]
    tab = nc.dram_tensor("tab", [128, NBLK * NCHK * DOUT], f32r,
                         kind="ExternalInput").ap()
    # cm[p, t*NCHK*128 + j*128 + m] = counts^T[chunk t][feature j*128+p, item m]
    cm = nc.dram_tensor("cm", [128, nch * NCHK * 128], f32r,
                        kind="ExternalInput").ap()
    out = nc.dram_tensor("out", [nch, 128, DOUT], f32,
                         kind="ExternalOutput").ap()

    with tile.TileContext(nc) as tc:
        with (
            tc.tile_pool(name="tabp", bufs=1) as tabp,
            tc.tile_pool(name="cmp", bufs=2) as cmp_,
            tc.tile_pool(name="outp", bufs=2) as outp,
            tc.tile_pool(name="ps", bufs=4, space="PSUM") as psp,
        ):
            # resident table tiles, one per block so deps are per-block
            tabt = []
            for b in range(NBLK):
                tt = tabp.tile([128, NCHK * DOUT], f32r, tag=f"tab{b}")
                nc.sync.dma_start(
                    tt[:], tab[:, b * NCHK * DOUT:(b + 1) * NCHK * DOUT])
                tabt.append(tt)

            for b in range(NBLK):
                # all count chunks for this block in one DMA (cpb*512KB)
                cmt = cmp_.tile([128, cpb * NCHK * 128], f32r, tag="cm")
                nc.sync.dma_start(
                    cmt[:],
                    cm[:, b * cpb * NCHK * 128:(b + 1) * cpb * NCHK * 128])
                outt = outp.tile([128, cpb, DOUT], f32, tag="out")
                for i in range(cpb):
                    ps = psp.tile([128, DOUT], f32, tag="ps")
                    for j in range(NCHK):
                        nc.tensor.matmul(
                            ps[:],
                            lhsT=cmt[:, (i * NCHK + j) * 128:(i * NCHK + j + 1) * 128],
                            rhs=tabt[b][:, j * DOUT:(j + 1) * DOUT],
                            start=(j == 0),
                            stop=(j == NCHK - 1),
                        )
                    # clip(psum, 0, 1) -> sbuf
                    nc.any.tensor_scalar(
                        outt[:, i, :], ps[:],
                        1.0, 0.0, mybir.AluOpType.min, mybir.AluOpType.max)
                nc.sync.dma_start(
                    out[b * cpb:(b + 1) * cpb].rearrange("t p d -> p t d"),
                    outt[:])

    nc.compile()
    return nc


def _prep(values, lengths, kings, mask):
    """Host prep: counts, mask-weighted factor sums, per-core item layout."""
    values = np.asarray(values).astype(np.int64)
    lengths = np.asarray(lengths).astype(np.int64)
    kings = np.asarray(kings).astype(np.int64)
    maskrows = np.asarray(mask, np.float32).reshape(64, PIECE)

    seg = np.repeat(np.arange(B, dtype=np.int64), lengths)

    # counts in merged-table column space; output b columns are flip-remapped
    cnt_a = np.bincount(seg * PIECE + values,
                        minlength=B * PIECE).reshape(B, PIECE)
    cnt_b = np.bincount(seg * PIECE + _FLIP_COL[values],
                        minlength=B * PIECE).reshape(B, PIECE)

    # block id per (output,bag) item, in merged-table space
    blk = np.concatenate([kings[:, 0], _PERM[kings[:, 1]]])

    ext = np.zeros((2 * B + 1, NFP), np.float32)  # last row stays zero (pad)
    cnt = ext[:2 * B, :PIECE]
    cnt[:B] = cnt_a
    cnt[B:] = cnt_b
    m = (cnt * maskrows[blk]).reshape(2 * B, KPL, 8, 8)
    ext[:2 * B, PIECE:PIECE + KPL] = m.sum(axis=(2, 3))
    ext[:2 * B, PIECE + KPL:PIECE + KPL + 96] = m.sum(axis=3).reshape(2 * B, 96)
    ext[:2 * B, PIECE + KPL + 96:NFEAT] = m.sum(axis=2).reshape(2 * B, 96)

    order = np.argsort(blk, kind="stable")
    nper = np.bincount(blk, minlength=64)
    offs = np.concatenate([[0], np.cumsum(nper)])
    cpb = max(5, int(np.ceil(nper.max() / 128)))
    nch = NBLK * cpb

    pad_idx = np.full((N_CORES, nch * 128), -1, np.int64)
    for t in range(64):
        core, slot = divmod(t, NBLK)
        ids = order[offs[t]:offs[t + 1]]
        base = slot * cpb * 128
        pad_idx[core, base:base + len(ids)] = ids

    return ext, pad_idx, cpb


def _make_tab(pieces, ranks, files, tiles):
    """[128, NBLK*NCHK*DOUT] per core: factor tables, never merged."""
    pieces = np.asarray(pieces, np.float32).reshape(64, KPL, DOUT)
    ranks = np.asarray(ranks, np.float32).reshape(64, KPL * 8, DOUT)
    files = np.asarray(files, np.float32).reshape(64, KPL * 8, DOUT)
    tiles = np.asarray(tiles, np.float32).reshape(64, PIECE, DOUT)
    big = np.zeros((64, NFP, DOUT), np.float32)
    big[:, :PIECE] = tiles
    big[:, PIECE:PIECE + KPL] = pieces
    big[:, PIECE + KPL:PIECE + KPL + 96] = ranks
    big[:, PIECE + KPL + 96:NFEAT] = files
    tabs = []
    for c in range(N_CORES):
        t = big[c * NBLK:(c + 1) * NBLK]          # [8, 1024, 256]
        t = t.reshape(NBLK, NCHK, 128, DOUT).transpose(2, 0, 1, 3)
        tabs.append(np.ascontiguousarray(t.reshape(128, -1)))
    return tabs


def _run(inputs, trace=False):
    ext, pad_idx, cpb = _prep(inputs["values"], inputs["lengths"],
                              inputs["kings"], inputs["factorization_mask"])
    nch = NBLK * cpb
    if cpb not in _prog_cache:
        _prog_cache[cpb] = _build_program(cpb)
    nc = _prog_cache[cpb]

    tabs = _make_tab(inputs["pieces"], inputs["ranks"], inputs["files"],
                     inputs["tiles"])

    in_maps = []
    for c in range(N_CORES):
        sel = ext[pad_idx[c]]                     # [nch*128, 1024]
        cmh = sel.reshape(nch, 128, NCHK, 128).transpose(3, 0, 2, 1)
        in_maps.append({
            "tab": tabs[c],
            "cm": np.ascontiguousarray(cmh.reshape(128, -1)),
        })

    res = run_bass_kernel_spmd(nc, in_maps, list(range(N_CORES)),
                               trace=trace)

    comb = np.zeros((2 * B, DOUT), np.float32)
    for c in range(N_CORES):
        flat = res.results[c]["out"].reshape(nch * 128, DOUT)
        valid = pad_idx[c] >= 0
        comb[pad_idx[c][valid]] = flat[valid]
    return (comb[:B], comb[B:]), res


def kernel(**inputs):
    (a, b), _ = _run(inputs, trace=False)
    return a, b


# revision 6
# speedup vs baseline: 1.2055x; 1.2055x over previous
"""Trainium2 kernel for the NNUE-style factorized embedding segment-sum.

Strategy: the ragged two-table embedding-bag is reformulated as block-diagonal
dense matmuls.  For each output row (bag), the gather+segment-sum over its
ragged feature ids equals  counts_row @ table_block, where table_block is the
768-row slice of the merged factorized table selected by the bag's king square
(and counts columns are flip-remapped for the second output so only ONE table
is ever needed).  The factorization (tiles + (pieces+ranks+files)*mask) is kept
factorized: counts are extended with mask-weighted per-(k), per-(k,rank) and
per-(k,file) sums so the device contracts against the raw input tables and
never materializes the merged table.

Host (integer work only): build per-bag count rows, group (output,bag) items by
table block, shard blocks over 8 cores.  Device (all fp work): per 128-item
chunk, accumulating matmuls (K=128, M=128, N=256) + clip to [0,1].

Default mode "hilo": tables are split into bf16 hi + bf16 lo residual and both
are contracted into the same fp32 PSUM (≈1e-5 rel err, full matmul speed);
counts ship as uint8 and are expanded to bf16 by a casting DMA on device.
Fallback mode "f32r" (fp32 tables, reduced-precision fast matmul) is used if
counts exceed uint8 range or the mask is not 0/1.
"""

import numpy as np
import ml_dtypes

import concourse.bass as bass
import concourse.tile as tile
from concourse import bacc, mybir
from concourse.bass_utils import run_bass_kernel_spmd

N_CORES = 8
B = 16384          # bags
KPL = 12           # piece planes
DOUT = 256
PIECE = 768        # KPL * 64
NFEAT = 972        # 768 tiles + 12 pieces + 96 ranks + 96 files
NFP = 1024         # padded features (8 chunks of 128)
NCHK = 8           # feature chunks per block
NBLK = 8           # table blocks per core (64 king squares / 8 cores)

# ---------------------------------------------------------------------------
# host-side integer prep tables
_sq = np.arange(64)
_PERM = (7 - _sq // 8) * 8 + _sq % 8          # vertical king-square flip
_v = np.arange(PIECE)
_vk, _vr, _vf = _v // 64, (_v % 64) // 8, _v % 8
_FLIP_COL = ((_vk + 6) % 12) * 64 + (7 - _vr) * 8 + _vf

_prog_cache = {}


def _build_program(cpb: int, mode: str):
    """Bass program for one core: NCH=8*cpb chunks of 128 output rows."""
    nch = NBLK * cpb
    nc = bacc.Bacc("TRN2", target_bir_lowering=False, debug=False)
    f32 = mybir.dt.float32
    npass = 2 if mode == "hilo" else 1
    tdt = {"hilo": mybir.dt.bfloat16, "f32r": mybir.dt.float32r}[mode]
    cdt = mybir.dt.uint8 if mode == "hilo" else tdt

    tabw = npass * NCHK * DOUT
    # tab[p, blk*tabw + (pass*NCHK+j)*DOUT + d] = table[blk,pass][j*128+p, d]
    tab = nc.dram_tensor("tab", [128, NBLK * tabw], tdt,
                         kind="ExternalInput").ap()
    # cm[p, t*NCHK*128 + j*128 + m] = counts^T[chunk t][feature j*128+p, item m]
    cm = nc.dram_tensor("cm", [128, nch * NCHK * 128], cdt,
                        kind="ExternalInput").ap()
    out = nc.dram_tensor("out", [nch, 128, DOUT], f32,
                         kind="ExternalOutput").ap()

    with tile.TileContext(nc) as tc:
        with (
            tc.tile_pool(name="tabp", bufs=1) as tabp,
            tc.tile_pool(name="cmp", bufs=2) as cmp_,
            tc.tile_pool(name="outp", bufs=2) as outp,
            tc.tile_pool(name="ps", bufs=4, space="PSUM") as psp,
        ):
            # resident table tiles, one per block so deps are per-block
            tabt = []
            for b in range(NBLK):
                tt = tabp.tile([128, tabw], tdt, tag=f"tab{b}")
                nc.sync.dma_start(tt[:], tab[:, b * tabw:(b + 1) * tabw])
                tabt.append(tt)

            cmw = cpb * NCHK * 128
            for b in range(NBLK):
                # all count chunks for this block in one DMA
                cmt = cmp_.tile([128, cmw], tdt, tag="cm")
                if mode == "hilo":
                    # uint8 -> bf16 cast during the (SWDGE) DMA
                    nc.gpsimd.dma_start(
                        cmt[:], cm[:, b * cmw:(b + 1) * cmw])
                else:
                    nc.sync.dma_start(
                        cmt[:], cm[:, b * cmw:(b + 1) * cmw])
                outt = outp.tile([128, cpb, DOUT], f32, tag="out")
                for i in range(cpb):
                    ps = psp.tile([128, DOUT], f32, tag="ps")
                    nmm = npass * NCHK
                    for q in range(nmm):
                        p_, j = divmod(q, NCHK)
                        nc.tensor.matmul(
                            ps[:],
                            lhsT=cmt[:, (i * NCHK + j) * 128:
                                     (i * NCHK + j + 1) * 128],
                            rhs=tabt[b][:, (p_ * NCHK + j) * DOUT:
                                        (p_ * NCHK + j + 1) * DOUT],
                            start=(q == 0),
                            stop=(q == nmm - 1),
                        )
                    # clip(psum, 0, 1) -> sbuf
                    nc.any.tensor_scalar(
                        outt[:, i, :], ps[:],
                        1.0, 0.0, mybir.AluOpType.min, mybir.AluOpType.max)
                nc.sync.dma_start(
                    out[b * cpb:(b + 1) * cpb].rearrange("t p d -> p t d"),
                    outt[:])

    nc.compile()
    return nc


def _prep(values, lengths, kings, mask):
    """Host prep: counts, mask-weighted factor sums, per-core item layout."""
    values = np.asarray(values).astype(np.int64)
    lengths = np.asarray(lengths).astype(np.int64)
    kings = np.asarray(kings).astype(np.int64)
    maskrows = np.asarray(mask, np.float32).reshape(64, PIECE)

    seg = np.repeat(np.arange(B, dtype=np.int64), lengths)

    # counts in merged-table column space; output b columns are flip-remapped
    cnt_a = np.bincount(seg * PIECE + values,
                        minlength=B * PIECE).reshape(B, PIECE)
    cnt_b = np.bincount(seg * PIECE + _FLIP_COL[values],
                        minlength=B * PIECE).reshape(B, PIECE)

    # block id per (output,bag) item, in merged-table space
    blk = np.concatenate([kings[:, 0], _PERM[kings[:, 1]]])

    ext = np.zeros((2 * B + 1, NFP), np.float32)  # last row stays zero (pad)
    cnt = ext[:2 * B, :PIECE]
    cnt[:B] = cnt_a
    cnt[B:] = cnt_b
    m = (cnt * maskrows[blk]).reshape(2 * B, KPL, 8, 8)
    ext[:2 * B, PIECE:PIECE + KPL] = m.sum(axis=(2, 3))
    ext[:2 * B, PIECE + KPL:PIECE + KPL + 96] = m.sum(axis=3).reshape(2 * B, 96)
    ext[:2 * B, PIECE + KPL + 96:NFEAT] = m.sum(axis=2).reshape(2 * B, 96)

    order = np.argsort(blk, kind="stable")
    nper = np.bincount(blk, minlength=64)
    offs = np.concatenate([[0], np.cumsum(nper)])
    cpb = max(5, int(np.ceil(nper.max() / 128)))
    nch = NBLK * cpb

    pad_idx = np.full((N_CORES, nch * 128), -1, np.int64)
    for t in range(64):
        core, slot = divmod(t, NBLK)
        ids = order[offs[t]:offs[t + 1]]
        base = slot * cpb * 128
        pad_idx[core, base:base + len(ids)] = ids

    # uint8-exact counts? (mask 0/1 and counts <= 255 -> "hilo" fast path)
    u8_ok = (np.all((maskrows == 0.0) | (maskrows == 1.0))
             and ext.max() <= 255.0)
    return ext, pad_idx, cpb, u8_ok


def _make_tab(pieces, ranks, files, tiles, mode):
    """Per-core [128, NBLK*npass*NCHK*DOUT]: factor tables, never merged."""
    pieces = np.asarray(pieces, np.float32).reshape(64, KPL, DOUT)
    ranks = np.asarray(ranks, np.float32).reshape(64, KPL * 8, DOUT)
    files = np.asarray(files, np.float32).reshape(64, KPL * 8, DOUT)
    tiles = np.asarray(tiles, np.float32).reshape(64, PIECE, DOUT)
    big = np.zeros((64, NFP, DOUT), np.float32)
    big[:, :PIECE] = tiles
    big[:, PIECE:PIECE + KPL] = pieces
    big[:, PIECE + KPL:PIECE + KPL + 96] = ranks
    big[:, PIECE + KPL + 96:NFEAT] = files

    if mode == "hilo":
        bf16 = ml_dtypes.bfloat16
        hi = big.astype(bf16)
        lo = (big - hi.astype(np.float32)).astype(bf16)
        # [64, 2, NCHK, 128, DOUT]
        planes = np.stack([hi, lo], axis=1).reshape(64, 2, NCHK, 128, DOUT)
    else:
        planes = big.reshape(64, 1, NCHK, 128, DOUT)

    tabs = []
    for c in range(N_CORES):
        t = planes[c * NBLK:(c + 1) * NBLK]    # [8, npass, NCHK, 128, DOUT]
        t = t.transpose(3, 0, 1, 2, 4)         # [128, blk, pass, chunk, dout]
        tabs.append(np.ascontiguousarray(t.reshape(128, -1)))
    return tabs


def _run(inputs, trace=False, force_mode=None):
    ext, pad_idx, cpb, u8_ok = _prep(
        inputs["values"], inputs["lengths"], inputs["kings"],
        inputs["factorization_mask"])
    mode = force_mode or ("hilo" if u8_ok else "f32r")
    nch = NBLK * cpb
    key = (cpb, mode)
    if key not in _prog_cache:
        _prog_cache[key] = _build_program(cpb, mode)
    nc = _prog_cache[key]

    tabs = _make_tab(inputs["pieces"], inputs["ranks"], inputs["files"],
                     inputs["tiles"], mode)

    cm_np_dtype = np.uint8 if mode == "hilo" else np.float32
    in_maps = []
    for c in range(N_CORES):
        sel = ext[pad_idx[c]]                  # [nch*128, 1024] f32
        cmh = sel.reshape(nch, 128, NCHK, 128).transpose(3, 0, 2, 1)
        in_maps.append({
            "tab": tabs[c],
            "cm": np.ascontiguousarray(cmh.reshape(128, -1)
                                       .astype(cm_np_dtype)),
        })

    res = run_bass_kernel_spmd(nc, in_maps, list(range(N_CORES)),
                               trace=trace)

    comb = np.zeros((2 * B, DOUT), np.float32)
    for c in range(N_CORES):
        flat = res.results[c]["out"].reshape(nch * 128, DOUT)
        valid = pad_idx[c] >= 0
        comb[pad_idx[c][valid]] = flat[valid]
    return (comb[:B], comb[B:]), res


def kernel(**inputs):
    (a, b), _ = _run(inputs, trace=False)
    return a, b


# revision 7
# speedup vs baseline: 1.2105x; 1.0041x over previous
"""Trainium2 kernel for the NNUE-style factorized embedding segment-sum.

Strategy: the ragged two-table embedding-bag is reformulated as block-diagonal
dense matmuls.  For each output row (bag), the gather+segment-sum over its
ragged feature ids equals  counts_row @ table_block, where table_block is the
768-row slice of the merged factorized table selected by the bag's king square
(and counts columns are flip-remapped for the second output so only ONE table
is ever needed).  The factorization (tiles + (pieces+ranks+files)*mask) is kept
factorized: counts are extended with mask-weighted per-(k), per-(k,rank) and
per-(k,file) sums so the device contracts against the raw input tables and
never materializes the merged table.

Host (integer work only): build per-bag count rows, group (output,bag) items by
table block, shard blocks over 8 cores.  Device (all fp work): per 128-item
chunk, accumulating matmuls (K=128, M=128, N=256) + clip to [0,1].

Default mode "hilo": tables are split into bf16 hi + bf16 lo residual and both
are contracted into the same fp32 PSUM (≈1e-5 rel err, full matmul speed);
counts ship as uint8 and are expanded to bf16 by a casting DMA on device.
Fallback mode "f32r" (fp32 tables, reduced-precision fast matmul) is used if
counts exceed uint8 range or the mask is not 0/1.
"""

import numpy as np
import ml_dtypes

import concourse.bass as bass
import concourse.tile as tile
from concourse import bacc, mybir
from concourse.bass_utils import run_bass_kernel_spmd

N_CORES = 8
B = 16384          # bags
KPL = 12           # piece planes
DOUT = 256
PIECE = 768        # KPL * 64
NFEAT = 972        # 768 tiles + 12 pieces + 96 ranks + 96 files
NFP = 1024         # padded features (8 chunks of 128)
NCHK = 8           # feature chunks per block
NBLK = 8           # table blocks per core (64 king squares / 8 cores)

# ---------------------------------------------------------------------------
# host-side integer prep tables
_sq = np.arange(64)
_PERM = (7 - _sq // 8) * 8 + _sq % 8          # vertical king-square flip
_v = np.arange(PIECE)
_vk, _vr, _vf = _v // 64, (_v % 64) // 8, _v % 8
_FLIP_COL = ((_vk + 6) % 12) * 64 + (7 - _vr) * 8 + _vf

_prog_cache = {}


def _build_program(cpb: int, mode: str):
    """Bass program for one core: NCH=8*cpb chunks of 128 output rows."""
    nch = NBLK * cpb
    nc = bacc.Bacc("TRN2", target_bir_lowering=False, debug=False)
    f32 = mybir.dt.float32
    npass = 2 if mode == "hilo" else 1
    tdt = {"hilo": mybir.dt.bfloat16, "f32r": mybir.dt.float32r}[mode]
    cdt = mybir.dt.uint8 if mode == "hilo" else tdt

    tabw = npass * NCHK * DOUT
    # tab[p, blk*tabw + (pass*NCHK+j)*DOUT + d] = table[blk,pass][j*128+p, d]
    tab = nc.dram_tensor("tab", [128, NBLK * tabw], tdt,
                         kind="ExternalInput").ap()
    # cm[p, t*NCHK*128 + j*128 + m] = counts^T[chunk t][feature j*128+p, item m]
    cm = nc.dram_tensor("cm", [128, nch * NCHK * 128], cdt,
                        kind="ExternalInput").ap()
    out = nc.dram_tensor("out", [nch, 128, DOUT], f32,
                         kind="ExternalOutput").ap()

    with tile.TileContext(nc) as tc:
        with (
            tc.tile_pool(name="tabp", bufs=1) as tabp,
            tc.tile_pool(name="cmup", bufs=1) as cmup,
            tc.tile_pool(name="cmp", bufs=3) as cmp_,
            tc.tile_pool(name="outp", bufs=2) as outp,
            tc.tile_pool(name="ps", bufs=4, space="PSUM") as psp,
        ):
            cmw = cpb * NCHK * 128
            # stage ALL uint8 count data upfront (small; decouples DMA
            # from compute), one resident tile per block
            cmu = []
            if mode == "hilo":
                u8 = mybir.dt.uint8
                for b in range(NBLK):
                    cu = cmup.tile([128, cmw], u8, tag=f"cmu{b}")
                    nc.sync.dma_start(cu[:], cm[:, b * cmw:(b + 1) * cmw])
                    cmu.append(cu)

            # resident table tiles, one per block so deps are per-block
            tabt = []
            for b in range(NBLK):
                tt = tabp.tile([128, tabw], tdt, tag=f"tab{b}")
                nc.sync.dma_start(tt[:], tab[:, b * tabw:(b + 1) * tabw])
                tabt.append(tt)

            for b in range(NBLK):
                cmt = cmp_.tile([128, cmw], tdt, tag="cm")
                if mode == "hilo":
                    # uint8 -> bf16 cast on DVE, split so it pipelines
                    half = cmw // 2
                    nc.vector.tensor_copy(cmt[:, :half], cmu[b][:, :half])
                    nc.vector.tensor_copy(cmt[:, half:], cmu[b][:, half:])
                else:
                    nc.sync.dma_start(
                        cmt[:], cm[:, b * cmw:(b + 1) * cmw])
                outt = outp.tile([128, cpb, DOUT], f32, tag="out")
                for i in range(cpb):
                    ps = psp.tile([128, DOUT], f32, tag="ps")
                    nmm = npass * NCHK
                    for q in range(nmm):
                        p_, j = divmod(q, NCHK)
                        nc.tensor.matmul(
                            ps[:],
                            lhsT=cmt[:, (i * NCHK + j) * 128:
                                     (i * NCHK + j + 1) * 128],
                            rhs=tabt[b][:, (p_ * NCHK + j) * DOUT:
                                        (p_ * NCHK + j + 1) * DOUT],
                            start=(q == 0),
                            stop=(q == nmm - 1),
                        )
                    # clip(psum, 0, 1) -> sbuf
                    nc.any.tensor_scalar(
                        outt[:, i, :], ps[:],
                        1.0, 0.0, mybir.AluOpType.min, mybir.AluOpType.max)
                nc.sync.dma_start(
                    out[b * cpb:(b + 1) * cpb].rearrange("t p d -> p t d"),
                    outt[:])

    nc.compile()
    return nc


def _prep(values, lengths, kings, mask):
    """Host prep: counts, mask-weighted factor sums, per-core item layout."""
    values = np.asarray(values).astype(np.int64)
    lengths = np.asarray(lengths).astype(np.int64)
    kings = np.asarray(kings).astype(np.int64)
    maskrows = np.asarray(mask, np.float32).reshape(64, PIECE)

    seg = np.repeat(np.arange(B, dtype=np.int64), lengths)

    # counts in merged-table column space; output b columns are flip-remapped
    cnt_a = np.bincount(seg * PIECE + values,
                        minlength=B * PIECE).reshape(B, PIECE)
    cnt_b = np.bincount(seg * PIECE + _FLIP_COL[values],
                        minlength=B * PIECE).reshape(B, PIECE)

    # block id per (output,bag) item, in merged-table space
    blk = np.concatenate([kings[:, 0], _PERM[kings[:, 1]]])

    ext = np.zeros((2 * B + 1, NFP), np.float32)  # last row stays zero (pad)
    cnt = ext[:2 * B, :PIECE]
    cnt[:B] = cnt_a
    cnt[B:] = cnt_b
    m = (cnt * maskrows[blk]).reshape(2 * B, KPL, 8, 8)
    ext[:2 * B, PIECE:PIECE + KPL] = m.sum(axis=(2, 3))
    ext[:2 * B, PIECE + KPL:PIECE + KPL + 96] = m.sum(axis=3).reshape(2 * B, 96)
    ext[:2 * B, PIECE + KPL + 96:NFEAT] = m.sum(axis=2).reshape(2 * B, 96)

    order = np.argsort(blk, kind="stable")
    nper = np.bincount(blk, minlength=64)
    offs = np.concatenate([[0], np.cumsum(nper)])
    cpb = max(5, int(np.ceil(nper.max() / 128)))
    nch = NBLK * cpb

    pad_idx = np.full((N_CORES, nch * 128), -1, np.int64)
    for t in range(64):
        core, slot = divmod(t, NBLK)
        ids = order[offs[t]:offs[t + 1]]
        base = slot * cpb * 128
        pad_idx[core, base:base + len(ids)] = ids

    # uint8-exact counts? (mask 0/1 and counts <= 255 -> "hilo" fast path)
    u8_ok = (np.all((maskrows == 0.0) | (maskrows == 1.0))
             and ext.max() <= 255.0)
    return ext, pad_idx, cpb, u8_ok


def _make_tab(pieces, ranks, files, tiles, mode):
    """Per-core [128, NBLK*npass*NCHK*DOUT]: factor tables, never merged."""
    pieces = np.asarray(pieces, np.float32).reshape(64, KPL, DOUT)
    ranks = np.asarray(ranks, np.float32).reshape(64, KPL * 8, DOUT)
    files = np.asarray(files, np.float32).reshape(64, KPL * 8, DOUT)
    tiles = np.asarray(tiles, np.float32).reshape(64, PIECE, DOUT)
    big = np.zeros((64, NFP, DOUT), np.float32)
    big[:, :PIECE] = tiles
    big[:, PIECE:PIECE + KPL] = pieces
    big[:, PIECE + KPL:PIECE + KPL + 96] = ranks
    big[:, PIECE + KPL + 96:NFEAT] = files

    if mode == "hilo":
        bf16 = ml_dtypes.bfloat16
        hi = big.astype(bf16)
        lo = (big - hi.astype(np.float32)).astype(bf16)
        # [64, 2, NCHK, 128, DOUT]
        planes = np.stack([hi, lo], axis=1).reshape(64, 2, NCHK, 128, DOUT)
    else:
        planes = big.reshape(64, 1, NCHK, 128, DOUT)

    tabs = []
    for c in range(N_CORES):
        t = planes[c * NBLK:(c + 1) * NBLK]    # [8, npass, NCHK, 128, DOUT]
        t = t.transpose(3, 0, 1, 2, 4)         # [128, blk, pass, chunk, dout]
        tabs.append(np.ascontiguousarray(t.reshape(128, -1)))
    return tabs


def _run(inputs, trace=False, force_mode=None):
    ext, pad_idx, cpb, u8_ok = _prep(
        inputs["values"], inputs["lengths"], inputs["kings"],
        inputs["factorization_mask"])
    mode = force_mode or ("hilo" if u8_ok else "f32r")
    nch = NBLK * cpb
    key = (cpb, mode)
    if key not in _prog_cache:
        _prog_cache[key] = _build_program(cpb, mode)
    nc = _prog_cache[key]

    tabs = _make_tab(inputs["pieces"], inputs["ranks"], inputs["files"],
                     inputs["tiles"], mode)

    cm_np_dtype = np.uint8 if mode == "hilo" else np.float32
    in_maps = []
    for c in range(N_CORES):
        sel = ext[pad_idx[c]]                  # [nch*128, 1024] f32
        cmh = sel.reshape(nch, 128, NCHK, 128).transpose(3, 0, 2, 1)
        in_maps.append({
            "tab": tabs[c],
            "cm": np.ascontiguousarray(cmh.reshape(128, -1)
                                       .astype(cm_np_dtype)),
        })

    res = run_bass_kernel_spmd(nc, in_maps, list(range(N_CORES)),
                               trace=trace)

    comb = np.zeros((2 * B, DOUT), np.float32)
    for c in range(N_CORES):
        flat = res.results[c]["out"].reshape(nch * 128, DOUT)
        valid = pad_idx[c] >= 0
        comb[pad_idx[c][valid]] = flat[valid]
    return (comb[:B], comb[B:]), res


def kernel(**inputs):
    (a, b), _ = _run(inputs, trace=False)
    return a, b


# revision 8
# speedup vs baseline: 1.4641x; 1.2095x over previous
"""Trainium2 kernel for the NNUE-style factorized embedding segment-sum.

Strategy: the ragged two-table embedding-bag is reformulated as block-diagonal
dense matmuls.  For each output row (bag), the gather+segment-sum over its
ragged feature ids equals  counts_row @ table_block, where table_block is the
768-row slice of the merged factorized table selected by the bag's king square
(and counts columns are flip-remapped for the second output so only ONE table
is ever needed).  The factorization (tiles + (pieces+ranks+files)*mask) is kept
factorized: counts are extended with mask-weighted per-(k), per-(k,rank) and
per-(k,file) sums so the device contracts against the raw input tables and
never materializes the merged table.

Host (integer work only): build per-bag count rows, group (output,bag) items by
table block, shard blocks over 8 cores.  Device (all fp work): per 128-item
chunk, accumulating matmuls (K=128, M=128, N=256) + clip to [0,1].

Default mode "hilo": tables are split into bf16 hi + bf16 lo residual and both
are contracted into the same fp32 PSUM (≈4e-5 rel err, full matmul speed);
counts ship as uint8 and are expanded to bf16 on the vector engine.  Fallback
mode "f32r" (fp32 tables, reduced-precision fast matmul) is used if counts
exceed uint8 range or the mask is not 0/1.

Blocks are assigned to (core, slot) so that each slot's chunk capacity (shared
across cores — the compiled program is SPMD) matches the data tightly.
"""

import numpy as np
import ml_dtypes

import concourse.bass as bass
import concourse.tile as tile
from concourse import bacc, mybir
from concourse.bass_utils import run_bass_kernel_spmd

N_CORES = 8
B = 16384          # bags
KPL = 12           # piece planes
DOUT = 256
PIECE = 768        # KPL * 64
NFEAT = 972        # 768 tiles + 12 pieces + 96 ranks + 96 files
NFP = 1024         # padded features (8 chunks of 128)
NCHK = 8           # feature chunks per block
NBLK = 8           # table blocks per core (64 king squares / 8 cores)

# ---------------------------------------------------------------------------
# host-side integer prep tables
_sq = np.arange(64)
_PERM = (7 - _sq // 8) * 8 + _sq % 8          # vertical king-square flip
_v = np.arange(PIECE)
_vk, _vr, _vf = _v // 64, (_v % 64) // 8, _v % 8
_FLIP_COL = ((_vk + 6) % 12) * 64 + (7 - _vr) * 8 + _vf

_prog_cache = {}


def _build_program(caps: tuple, mode: str):
    """Bass program for one core.

    caps[s] = number of 128-item chunks for block slot s (shared by all
    cores).  Per slot: DMA table block + counts, cast counts, then per chunk
    npass*NCHK accumulating matmuls and a clipped PSUM->SBUF->HBM drain.
    """
    nch = sum(caps)
    nc = bacc.Bacc("TRN2", target_bir_lowering=False, debug=False)
    f32 = mybir.dt.float32
    npass = 2 if mode == "hilo" else 1
    tdt = {"hilo": mybir.dt.bfloat16, "f32r": mybir.dt.float32r}[mode]
    cdt = mybir.dt.uint8 if mode == "hilo" else tdt

    tabw = npass * NCHK * DOUT
    # tab[p, blk*tabw + (pass*NCHK+j)*DOUT + d] = table[blk,pass][j*128+p, d]
    tab = nc.dram_tensor("tab", [128, NBLK * tabw], tdt,
                         kind="ExternalInput").ap()
    # cm[p, (chunkbase(s)+i)*NCHK*128 + j*128 + m]
    #    = counts^T[slot s, chunk i][feature j*128+p, item m]
    cm = nc.dram_tensor("cm", [128, nch * NCHK * 128], cdt,
                        kind="ExternalInput").ap()
    out = nc.dram_tensor("out", [nch, 128, DOUT], f32,
                         kind="ExternalOutput").ap()

    cbase = np.concatenate([[0], np.cumsum(caps)]).astype(int)
    maxw = max(caps) * NCHK * 128

    with tile.TileContext(nc) as tc:
        with (
            tc.tile_pool(name="tabp", bufs=3) as tabp,
            tc.tile_pool(name="cmup", bufs=3) as cmup,
            tc.tile_pool(name="cmp", bufs=3) as cmp_,
            tc.tile_pool(name="outp", bufs=4) as outp,
            tc.tile_pool(name="ps", bufs=4, space="PSUM") as psp,
        ):
            for b in range(NBLK):
                cmw = caps[b] * NCHK * 128
                if mode == "hilo":
                    cu = cmup.tile([128, maxw], mybir.dt.uint8, tag="cmu")
                    nc.sync.dma_start(
                        cu[:, :cmw],
                        cm[:, cbase[b] * NCHK * 128:cbase[b + 1] * NCHK * 128])
                tt = tabp.tile([128, tabw], tdt, tag="tab")
                nc.sync.dma_start(tt[:], tab[:, b * tabw:(b + 1) * tabw])
                cmt = cmp_.tile([128, maxw], tdt, tag="cm")
                if mode == "hilo":
                    # uint8 -> bf16 cast on DVE, split so it pipelines
                    half = (cmw // 2) // 128 * 128
                    nc.vector.tensor_copy(cmt[:, :half], cu[:, :half])
                    nc.vector.tensor_copy(cmt[:, half:cmw], cu[:, half:cmw])
                else:
                    nc.sync.dma_start(
                        cmt[:, :cmw],
                        cm[:, cbase[b] * NCHK * 128:cbase[b + 1] * NCHK * 128])

                for i in range(caps[b]):
                    ps = psp.tile([128, DOUT], f32, tag="ps")
                    nmm = npass * NCHK
                    for q in range(nmm):
                        p_, j = divmod(q, NCHK)
                        nc.tensor.matmul(
                            ps[:],
                            lhsT=cmt[:, (i * NCHK + j) * 128:
                                     (i * NCHK + j + 1) * 128],
                            rhs=tt[:, (p_ * NCHK + j) * DOUT:
                                   (p_ * NCHK + j + 1) * DOUT],
                            start=(q == 0),
                            stop=(q == nmm - 1),
                        )
                    # clip(psum, 0, 1) -> sbuf -> HBM (per chunk)
                    outt = outp.tile([128, DOUT], f32, tag="out")
                    nc.any.tensor_scalar(
                        outt[:], ps[:],
                        1.0, 0.0, mybir.AluOpType.min, mybir.AluOpType.max)
                    nc.sync.dma_start(
                        out[cbase[b] + i].rearrange("p d -> p d"), outt[:])

    nc.compile()
    return nc


def _prep(values, lengths, kings, mask):
    """Host prep: counts, mask-weighted factor sums, per-core item layout."""
    values = np.asarray(values).astype(np.int64)
    lengths = np.asarray(lengths).astype(np.int64)
    kings = np.asarray(kings).astype(np.int64)
    maskrows = np.asarray(mask, np.float32).reshape(64, PIECE)

    seg = np.repeat(np.arange(B, dtype=np.int64), lengths)

    # counts in merged-table column space; output b columns are flip-remapped
    cnt_a = np.bincount(seg * PIECE + values,
                        minlength=B * PIECE).reshape(B, PIECE)
    cnt_b = np.bincount(seg * PIECE + _FLIP_COL[values],
                        minlength=B * PIECE).reshape(B, PIECE)

    # block id per (output,bag) item, in merged-table space
    blk = np.concatenate([kings[:, 0], _PERM[kings[:, 1]]])

    ext = np.zeros((2 * B + 1, NFP), np.float32)  # last row stays zero (pad)
    cnt = ext[:2 * B, :PIECE]
    cnt[:B] = cnt_a
    cnt[B:] = cnt_b
    m = (cnt * maskrows[blk]).reshape(2 * B, KPL, 8, 8)
    ext[:2 * B, PIECE:PIECE + KPL] = m.sum(axis=(2, 3))
    ext[:2 * B, PIECE + KPL:PIECE + KPL + 96] = m.sum(axis=3).reshape(2 * B, 96)
    ext[:2 * B, PIECE + KPL + 96:NFEAT] = m.sum(axis=2).reshape(2 * B, 96)

    order = np.argsort(blk, kind="stable")
    nper = np.bincount(blk, minlength=64)
    offs = np.concatenate([[0], np.cumsum(nper)])
    nchunks = np.maximum(np.ceil(nper / 128).astype(int), 1)

    # assign blocks to (core, slot): sort by descending chunk need so each
    # slot's shared capacity is tight
    rank = np.argsort(-nchunks, kind="stable")      # block ids, desc need
    caps = tuple(int(nchunks[rank[s * N_CORES]]) for s in range(NBLK))
    cbase = np.concatenate([[0], np.cumsum(caps)]).astype(int)
    nch = int(cbase[-1])

    pad_idx = np.full((N_CORES, nch * 128), -1, np.int64)
    for s in range(NBLK):
        for c in range(N_CORES):
            t = rank[s * N_CORES + c]               # block for (core c, slot s)
            ids = order[offs[t]:offs[t + 1]]
            base = cbase[s] * 128
            pad_idx[c, base:base + len(ids)] = ids

    # block index (0..63) per (core, slot), for table selection
    blk_of = rank.reshape(NBLK, N_CORES).T          # [core, slot]

    # uint8-exact counts? (mask 0/1 and counts <= 255 -> "hilo" fast path)
    u8_ok = (np.all((maskrows == 0.0) | (maskrows == 1.0))
             and ext.max() <= 255.0)
    return ext, pad_idx, caps, blk_of, u8_ok


def _make_tab(pieces, ranks, files, tiles, blk_of, mode):
    """Per-core [128, NBLK*npass*NCHK*DOUT]: factor tables, never merged."""
    pieces = np.asarray(pieces, np.float32).reshape(64, KPL, DOUT)
    ranks = np.asarray(ranks, np.float32).reshape(64, KPL * 8, DOUT)
    files = np.asarray(files, np.float32).reshape(64, KPL * 8, DOUT)
    tiles = np.asarray(tiles, np.float32).reshape(64, PIECE, DOUT)
    big = np.zeros((64, NFP, DOUT), np.float32)
    big[:, :PIECE] = tiles
    big[:, PIECE:PIECE + KPL] = pieces
    big[:, PIECE + KPL:PIECE + KPL + 96] = ranks
    big[:, PIECE + KPL + 96:NFEAT] = files

    if mode == "hilo":
        bf16 = ml_dtypes.bfloat16
        hi = big.astype(bf16)
        lo = (big - hi.astype(np.float32)).astype(bf16)
        # [64, npass, NCHK, 128, DOUT]
        planes = np.stack([hi, lo], axis=1).reshape(64, 2, NCHK, 128, DOUT)
    else:
        planes = big.reshape(64, 1, NCHK, 128, DOUT)

    tabs = []
    for c in range(N_CORES):
        t = planes[blk_of[c]]                  # [8, npass, NCHK, 128, DOUT]
        t = t.transpose(3, 0, 1, 2, 4)         # [128, slot, pass, chunk, dout]
        tabs.append(np.ascontiguousarray(t.reshape(128, -1)))
    return tabs


def _run(inputs, trace=False, force_mode=None):
    ext, pad_idx, caps, blk_of, u8_ok = _prep(
        inputs["values"], inputs["lengths"], inputs["kings"],
        inputs["factorization_mask"])
    mode = force_mode or ("hilo" if u8_ok else "f32r")
    nch = sum(caps)
    key = (caps, mode)
    if key not in _prog_cache:
        _prog_cache[key] = _build_program(caps, mode)
    nc = _prog_cache[key]

    tabs = _make_tab(inputs["pieces"], inputs["ranks"], inputs["files"],
                     inputs["tiles"], blk_of, mode)

    cm_np_dtype = np.uint8 if mode == "hilo" else np.float32
    in_maps = []
    for c in range(N_CORES):
        sel = ext[pad_idx[c]]                  # [nch*128, 1024] f32
        cmh = sel.reshape(nch, 128, NCHK, 128).transpose(3, 0, 2, 1)
        in_maps.append({
            "tab": tabs[c],
            "cm": np.ascontiguousarray(cmh.reshape(128, -1)
                                       .astype(cm_np_dtype)),
        })

    res = run_bass_kernel_spmd(nc, in_maps, list(range(N_CORES)),
                               trace=trace)

    comb = np.zeros((2 * B, DOUT), np.float32)
    for c in range(N_CORES):
        flat = res.results[c]["out"].reshape(nch * 128, DOUT)
        valid = pad_idx[c] >= 0
        comb[pad_idx[c][valid]] = flat[valid]
    return (comb[:B], comb[B:]), res


def kernel(**inputs):
    (a, b), _ = _run(inputs, trace=False)
    return a, b
